# revision 9
# baseline (speedup 1.0000x reference)
"""Multi-head attention (B=2, N=2048, C=1024, H=16) on 8 trn2 NeuronCores.

Strategy: tensor-parallel over heads. Core c computes heads {2c, 2c+1} for
both batch elements:
  - QKV^T projection from pre-transposed x (x^T supplied by host, bf16)
  - S^T = K @ Q^T, both local heads via a zero-padded Q^T layout: QP[b] is
    [128, 2*SEQ]; block h holds Q^T_h in its own 64 rows and zeros in the
    other head's rows, so the stacked-stationary K^T ([128, k] = both
    heads) contracts cleanly per block at full K=128 rate (K=64 matmuls
    measured 424ns vs 222ns for K=128).
  - P^T = exp(S^T / 32) on ScalarE directly from PSUM (no max subtraction:
    |scores/32| <~ 1.5 so exp is safe in fp32)
  - PV via ones-augmented V (65th stationary column accumulates softmax
    denominators for free); normalization = ones-broadcast matmul +
    fast approximate reciprocal + DVE multiply straight from PSUM
  - partial y_c = attn_out_c @ W_out[rows of local heads] in fp32r
Host: y = sum_c y_c + b_out.

Attention operands are bf16 (error ~2.5e-3 of output scale vs the fp32
reference); the output projection runs in fp32r (fp32 rounded to 11
mantissa bits, full PE rate).
"""
import sys

sys.path.insert(0, "/opt/trn_rl_repo")

import ml_dtypes
import numpy as np

import concourse.bacc as bacc
import concourse.mybir as mybir
import concourse.tile as tile
from concourse import bass_utils
from concourse.masks import make_identity

F32 = mybir.dt.float32
F32R = mybir.dt.float32r
BF16 = mybir.dt.bfloat16
NPBF16 = ml_dtypes.bfloat16

EMB = 1024
HEADS = 16
B = 2
SEQ = 2048
D = 64
NCORES = 8
HPC = HEADS // NCORES          # heads per core = 2
LD = HPC * D                   # local head dim = 128
TSEQ = B * SEQ                 # 4096
CC = EMB // 128                # contraction chunks = 8
SCALE = float(EMB) ** -0.5     # 1/32

QCH = 512                      # q chunk (free dim of S^T matmuls)
NQ = SEQ // QCH                # 4 q-chunks per batch
NK = SEQ // 128                # 16 k-chunks per batch


def _round_fp32r(x: np.ndarray) -> np.ndarray:
    bits = np.ascontiguousarray(x, dtype=np.float32).view(np.uint32)
    out = ((bits.astype(np.uint64) + 0x800) & 0xFFFFF000).astype(np.uint32)
    return out.view(np.float32)


def _build():
    nc = bacc.Bacc("TRN2", target_bir_lowering=False, debug=False,
                   num_devices=NCORES)

    xT = nc.dram_tensor("xT", [EMB, TSEQ], BF16, kind="ExternalInput")
    wq = nc.dram_tensor("wq", [EMB, LD], BF16, kind="ExternalInput")
    wk = nc.dram_tensor("wk", [EMB, LD], BF16, kind="ExternalInput")
    wv = nc.dram_tensor("wv", [EMB, LD], BF16, kind="ExternalInput")
    bq = nc.dram_tensor("bq", [LD, 1], F32, kind="ExternalInput")
    bk = nc.dram_tensor("bk", [LD, 1], F32, kind="ExternalInput")
    bv = nc.dram_tensor("bv", [LD, 1], F32, kind="ExternalInput")
    wout = nc.dram_tensor("wout", [LD, EMB], F32R, kind="ExternalInput")
    ones = nc.dram_tensor("ones", [1, D], F32R, kind="ExternalInput")
    onescol = nc.dram_tensor("onescol", [128, 1], F32R, kind="ExternalInput")
    y = nc.dram_tensor("y", [TSEQ, EMB], F32, kind="ExternalOutput")

    xT_c = xT.ap().rearrange("(kc p) s -> kc p s", p=128)
    wq_c = wq.ap().rearrange("(kc p) m -> kc p m", p=128)
    wk_c = wk.ap().rearrange("(kc p) m -> kc p m", p=128)
    wv_c = wv.ap().rearrange("(kc p) m -> kc p m", p=128)

    with tile.TileContext(nc) as tc:
        with (
            tc.tile_pool(name="persist", bufs=1) as persist,
            tc.tile_pool(name="xt", bufs=2) as xtp,
            tc.tile_pool(name="vt", bufs=2) as vtp,
            tc.tile_pool(name="psb", bufs=3) as psb,
            tc.tile_pool(name="norm", bufs=2) as normp,
            tc.tile_pool(name="yout", bufs=6) as youtp,
            tc.tile_pool(name="ps_st", bufs=2, space="PSUM") as ps_st,
            tc.tile_pool(name="ps_pv", bufs=1, space="PSUM") as ps_pv,
            tc.tile_pool(name="ps_misc", bufs=2, space="PSUM") as ps_misc,
        ):
            # ---- constants / weights ----
            bq_sb = persist.tile([LD, 1], F32, tag="bq")
            bk_sb = persist.tile([LD, 1], F32, tag="bk")
            bv_sb = persist.tile([LD, 1], F32, tag="bv")
            nc.sync.dma_start(bq_sb[:], bq.ap())
            nc.sync.dma_start(bk_sb[:], bk.ap())
            nc.sync.dma_start(bv_sb[:], bv.ap())
            w_sb = {}
            for nm, c_ap in (("q", wq_c), ("k", wk_c), ("v", wv_c)):
                for kc in range(CC):
                    t = persist.tile([128, LD], BF16, tag=f"w{nm}{kc}")
                    nc.sync.dma_start(t[:], c_ap[kc])
                    w_sb[nm, kc] = t
            ident = persist.tile([128, 128], BF16, tag="ident")
            make_identity(nc, ident[:])
            ones_sb = persist.tile([1, D], F32R, tag="ones")
            nc.sync.dma_start(ones_sb[:], ones.ap())
            onescol_sb = persist.tile([128, 1], F32R, tag="onescol")
            nc.sync.dma_start(onescol_sb[:], onescol.ap())
            wout_sb = persist.tile([LD, EMB], F32R, tag="wout")

            # persistent activations (per batch)
            # QP: zero-padded Q^T. Block h occupies cols [h*SEQ, (h+1)*SEQ):
            # rows [h*64,(h+1)*64) hold Q^T_h, the other 64 rows are zero.
            QP = [persist.tile([128, HPC * SEQ], BF16, tag=f"QP{b}",
                               name=f"QP{b}") for b in range(B)]
            KT = [persist.tile([LD, SEQ], BF16, tag=f"KT{b}", name=f"KT{b}")
                  for b in range(B)]
            outT = [persist.tile([LD, SEQ], F32R, tag=f"outT{b}",
                                 name=f"outT{b}") for b in range(B)]
            vaug = {}  # (b, kc) -> [128, 2*(D+1)] tile
            for b in range(B):
                for kc in range(NK):
                    vaug[b, kc] = persist.tile([128, 2 * (D + 1)], BF16,
                                               tag=f"vaug{b}_{kc}",
                                               name=f"vaug{b}_{kc}")

            def phase_a(b):
                """QKV^T projection + V transpose for batch b."""
                # zero quadrants of QP (once per batch; stay zero)
                nc.vector.memset(QP[b][D:2 * D, 0:SEQ], 0.0)
                nc.vector.memset(QP[b][0:D, SEQ:2 * SEQ], 0.0)
                xts = None
                for sc in range(SEQ // 512):
                    s0 = sc * 512            # batch-local seq offset
                    g0 = b * SEQ + s0        # global column in xT
                    if sc % 2 == 0:
                        xts = []
                        for kc in range(CC):
                            t = xtp.tile([128, 1024], BF16, tag=f"xt{kc}")
                            nc.sync.dma_start(t[:], xT_c[kc, :, g0:g0 + 1024])
                            xts.append(t)
                    xo = (sc % 2) * 512
                    for nm in ("q", "k", "v"):
                        ps = ps_misc.tile([128, 512], F32, tag="misc")
                        for kc in range(CC):
                            nc.tensor.matmul(ps[:], w_sb[nm, kc][:],
                                             xts[kc][:, xo:xo + 512],
                                             start=(kc == 0), stop=(kc == CC - 1))
                        if nm == "q":
                            for h in range(HPC):
                                nc.vector.tensor_scalar_add(
                                    QP[b][h * D:(h + 1) * D,
                                          h * SEQ + s0:h * SEQ + s0 + 512],
                                    ps[h * D:(h + 1) * D, :],
                                    bq_sb[h * D:(h + 1) * D, :])
                        elif nm == "k":
                            nc.vector.tensor_scalar_add(
                                KT[b][:, s0:s0 + 512], ps[:], bk_sb[:])
                        else:
                            vt = vtp.tile([128, 512], BF16, tag="vt")
                            nc.vector.tensor_scalar_add(vt[:], ps[:], bv_sb[:])
                            # transpose V^T -> V (natural), split per head with
                            # a ones column appended after each head's 64 cols
                            for j in range(4):
                                va = vaug[b, sc * 4 + j]
                                pst = ps_misc.tile([128, 128], BF16, tag="misc")
                                nc.tensor.transpose(
                                    pst[:], vt[:, j * 128:(j + 1) * 128],
                                    ident[:])
                                nc.vector.tensor_copy(va[:, 0:D], pst[:, 0:D])
                                nc.vector.tensor_copy(va[:, D + 1:2 * D + 1],
                                                      pst[:, D:2 * D])
                                nc.vector.tensor_copy(va[:, D:D + 1],
                                                      onescol_sb[:])
                                nc.vector.tensor_copy(
                                    va[:, 2 * D + 1:2 * D + 2], onescol_sb[:])

            def phase_bc(b):
                """Attention + output projection for batch b, interleaved
                per q-chunk so projection fills exp-gated PE slack."""
                for q in range(NQ):
                    q0 = q * QCH
                    pvs = [ps_pv.tile([D + 1, QCH], F32, tag=f"pv{h}",
                                      name=f"pv{h}") for h in range(HPC)]
                    for kc in range(NK):
                        st = ps_st.tile([128, 2 * QCH], F32, tag="st")
                        k0 = kc * 128
                        for h in range(HPC):
                            nc.tensor.matmul(
                                st[:, h * QCH:(h + 1) * QCH],
                                KT[b][:, k0:k0 + 128],
                                QP[b][:, h * SEQ + q0:h * SEQ + q0 + QCH],
                                start=True, stop=True)
                        pt = psb.tile([128, 2 * QCH], BF16, tag="pt")
                        nc.scalar.activation(pt[:], st[:],
                                             mybir.ActivationFunctionType.Exp,
                                             scale=SCALE)
                        for h in range(HPC):
                            nc.tensor.matmul(
                                pvs[h][:],
                                vaug[b, kc][:, h * (D + 1):(h + 1) * (D + 1)],
                                pt[:, h * QCH:(h + 1) * QCH],
                                start=(kc == 0), stop=(kc == NK - 1))
                    # normalize: out^T[d, q] / colsum -> outT (fp32r).
                    for h in range(HPC):
                        ss = normp.tile([1, QCH], F32R, tag="ss")
                        nc.vector.tensor_copy(ss[:], pvs[h][D:D + 1, :])
                        bc = ps_misc.tile([D, QCH], F32, tag="misc")
                        nc.tensor.matmul(bc[:], ones_sb[:], ss[:],
                                         start=True, stop=True)
                        rc = normp.tile([D, QCH], F32, tag="rc")
                        nc.vector.reciprocal_approx_fast(rc[:], bc[:])
                        nc.vector.tensor_mul(
                            outT[b][h * D:(h + 1) * D, q0:q0 + QCH],
                            pvs[h][0:D, :], rc[:])
                    # projection for the 4 seq-128 chunks of this q-chunk
                    for sc in range(4 * q, 4 * q + 4):
                        r0 = b * SEQ + sc * 128
                        for n in range(EMB // 512):
                            ps = ps_misc.tile([128, 512], F32, tag="misc")
                            nc.tensor.matmul(
                                ps[:], outT[b][:, sc * 128:(sc + 1) * 128],
                                wout_sb[:, n * 512:(n + 1) * 512],
                                start=True, stop=True)
                            yt = youtp.tile([128, 512], F32, tag="yt")
                            nc.vector.tensor_copy(yt[:], ps[:])
                            nc.gpsimd.dma_start(
                                y.ap()[r0:r0 + 128, n * 512:(n + 1) * 512],
                                yt[:])

            phase_a(0)
            nc.sync.dma_start(wout_sb[:], wout.ap())
            phase_bc(0)
            phase_a(1)
            phase_bc(1)

    nc.compile()
    return nc


_NC = None


def _get_nc():
    global _NC
    if _NC is None:
        _NC = _build()
    return _NC


def kernel(x, W_qkv, b_qkv, W_out, b_out):
    x = np.asarray(x, dtype=np.float32)
    W_qkv = np.asarray(W_qkv, dtype=np.float32)
    b_qkv = np.asarray(b_qkv, dtype=np.float32)
    W_out = np.asarray(W_out, dtype=np.float32)
    b_out = np.asarray(b_out, dtype=np.float32)

    nc = _get_nc()

    xT = np.ascontiguousarray(x.reshape(TSEQ, EMB).T).astype(NPBF16)
    Wr = W_qkv.reshape(EMB, 3, HEADS, D)
    br = b_qkv.reshape(3, HEADS, D)
    ones = np.ones((1, D), dtype=np.float32)
    onescol = np.ones((128, 1), dtype=np.float32)

    in_maps = []
    for c in range(NCORES):
        h0, h1 = HPC * c, HPC * (c + 1)
        in_maps.append({
            "xT": xT,
            "wq": np.ascontiguousarray(
                Wr[:, 0, h0:h1].reshape(EMB, LD)).astype(NPBF16),
            "wk": np.ascontiguousarray(
                Wr[:, 1, h0:h1].reshape(EMB, LD)).astype(NPBF16),
            "wv": np.ascontiguousarray(
                Wr[:, 2, h0:h1].reshape(EMB, LD)).astype(NPBF16),
            "bq": np.ascontiguousarray(br[0, h0:h1].reshape(LD, 1)),
            "bk": np.ascontiguousarray(br[1, h0:h1].reshape(LD, 1)),
            "bv": np.ascontiguousarray(br[2, h0:h1].reshape(LD, 1)),
            "wout": _round_fp32r(W_out[LD * c:LD * (c + 1)]),
            "ones": ones,
            "onescol": onescol,
        })

    res = bass_utils.run_bass_kernel_spmd(
        nc, in_maps, core_ids=list(range(NCORES)), trace=False)

    acc = np.zeros((TSEQ, EMB), dtype=np.float64)
    for c in range(NCORES):
        acc += res.results[c]["y"]
    out = (acc + b_out).astype(np.float32)
    return out.reshape(B, SEQ, EMB)


# revision 10
# speedup vs baseline: 1.0014x; 1.0014x over previous
"""Multi-head attention (B=2, N=2048, C=1024, H=16) on 8 trn2 NeuronCores.

Strategy: tensor-parallel over heads. Core c computes heads {2c, 2c+1} for
both batch elements:
  - QKV^T projection from pre-transposed x (x^T supplied by host, bf16)
  - S^T = K @ Q^T, both local heads via a zero-padded Q^T layout: QP[b] is
    [128, 2*SEQ]; block h holds Q^T_h in its own 64 rows and zeros in the
    other head's rows, so the stacked-stationary K^T ([128, k] = both
    heads) contracts cleanly per block at full K=128 rate (K=64 matmuls
    measured 424ns vs 222ns for K=128).
  - P^T = exp(S^T / 32) on ScalarE directly from PSUM (no max subtraction:
    |scores/32| <~ 1.5 so exp is safe in fp32)
  - PV via ones-augmented V (65th stationary column accumulates softmax
    denominators for free); normalization = ones-broadcast matmul +
    fast approximate reciprocal + DVE multiply straight from PSUM
  - partial y_c = attn_out_c @ W_out[rows of local heads] in fp32r
Host: y = sum_c y_c + b_out.

Attention operands are bf16 (error ~2.5e-3 of output scale vs the fp32
reference); the output projection runs in fp32r (fp32 rounded to 11
mantissa bits, full PE rate).
"""
import sys

sys.path.insert(0, "/opt/trn_rl_repo")

import ml_dtypes
import numpy as np

import concourse.bacc as bacc
import concourse.mybir as mybir
import concourse.tile as tile
from concourse import bass_utils
from concourse.masks import make_identity

F32 = mybir.dt.float32
F32R = mybir.dt.float32r
BF16 = mybir.dt.bfloat16
NPBF16 = ml_dtypes.bfloat16

EMB = 1024
HEADS = 16
B = 2
SEQ = 2048
D = 64
NCORES = 8
HPC = HEADS // NCORES          # heads per core = 2
LD = HPC * D                   # local head dim = 128
TSEQ = B * SEQ                 # 4096
CC = EMB // 128                # contraction chunks = 8
SCALE = float(EMB) ** -0.5     # 1/32

QCH = 512                      # q chunk (free dim of S^T matmuls)
NQ = SEQ // QCH                # 4 q-chunks per batch
NK = SEQ // 128                # 16 k-chunks per batch


def _round_fp32r(x: np.ndarray) -> np.ndarray:
    bits = np.ascontiguousarray(x, dtype=np.float32).view(np.uint32)
    out = ((bits.astype(np.uint64) + 0x800) & 0xFFFFF000).astype(np.uint32)
    return out.view(np.float32)


def _build():
    nc = bacc.Bacc("TRN2", target_bir_lowering=False, debug=False,
                   num_devices=NCORES)

    xT = nc.dram_tensor("xT", [CC, TSEQ // 1024, 128, 1024], BF16,
                        kind="ExternalInput")
    wq = nc.dram_tensor("wq", [EMB, LD], BF16, kind="ExternalInput")
    wk = nc.dram_tensor("wk", [EMB, LD], BF16, kind="ExternalInput")
    wv = nc.dram_tensor("wv", [EMB, LD], BF16, kind="ExternalInput")
    bq = nc.dram_tensor("bq", [LD, 1], F32, kind="ExternalInput")
    bk = nc.dram_tensor("bk", [LD, 1], F32, kind="ExternalInput")
    bv = nc.dram_tensor("bv", [LD, 1], F32, kind="ExternalInput")
    wout = nc.dram_tensor("wout", [LD, EMB], F32R, kind="ExternalInput")
    ones = nc.dram_tensor("ones", [1, D], F32R, kind="ExternalInput")
    onescol = nc.dram_tensor("onescol", [128, 1], F32R, kind="ExternalInput")
    y = nc.dram_tensor("y", [TSEQ // 128, EMB // 512, 128, 512], F32,
                       kind="ExternalOutput")

    xT_c = xT.ap()
    wq_c = wq.ap().rearrange("(kc p) m -> kc p m", p=128)
    wk_c = wk.ap().rearrange("(kc p) m -> kc p m", p=128)
    wv_c = wv.ap().rearrange("(kc p) m -> kc p m", p=128)

    with tile.TileContext(nc) as tc:
        with (
            tc.tile_pool(name="persist", bufs=1) as persist,
            tc.tile_pool(name="xt", bufs=2) as xtp,
            tc.tile_pool(name="vt", bufs=2) as vtp,
            tc.tile_pool(name="psb", bufs=3) as psb,
            tc.tile_pool(name="norm", bufs=2) as normp,
            tc.tile_pool(name="yout", bufs=6) as youtp,
            tc.tile_pool(name="ps_st", bufs=2, space="PSUM") as ps_st,
            tc.tile_pool(name="ps_pv", bufs=1, space="PSUM") as ps_pv,
            tc.tile_pool(name="ps_misc", bufs=2, space="PSUM") as ps_misc,
        ):
            # ---- constants / weights ----
            bq_sb = persist.tile([LD, 1], F32, tag="bq")
            bk_sb = persist.tile([LD, 1], F32, tag="bk")
            bv_sb = persist.tile([LD, 1], F32, tag="bv")
            nc.sync.dma_start(bq_sb[:], bq.ap())
            nc.sync.dma_start(bk_sb[:], bk.ap())
            nc.sync.dma_start(bv_sb[:], bv.ap())
            w_sb = {}
            for nm, c_ap in (("q", wq_c), ("k", wk_c), ("v", wv_c)):
                for kc in range(CC):
                    t = persist.tile([128, LD], BF16, tag=f"w{nm}{kc}")
                    nc.sync.dma_start(t[:], c_ap[kc])
                    w_sb[nm, kc] = t
            ident = persist.tile([128, 128], BF16, tag="ident")
            make_identity(nc, ident[:])
            ones_sb = persist.tile([1, D], F32R, tag="ones")
            nc.sync.dma_start(ones_sb[:], ones.ap())
            onescol_sb = persist.tile([128, 1], F32R, tag="onescol")
            nc.sync.dma_start(onescol_sb[:], onescol.ap())
            wout_sb = persist.tile([LD, EMB], F32R, tag="wout")

            # persistent activations (per batch)
            # QP: zero-padded Q^T. Block h occupies cols [h*SEQ, (h+1)*SEQ):
            # rows [h*64,(h+1)*64) hold Q^T_h, the other 64 rows are zero.
            QP = [persist.tile([128, HPC * SEQ], BF16, tag=f"QP{b}",
                               name=f"QP{b}") for b in range(B)]
            KT = [persist.tile([LD, SEQ], BF16, tag=f"KT{b}", name=f"KT{b}")
                  for b in range(B)]
            outT = [persist.tile([LD, SEQ], F32R, tag=f"outT{b}",
                                 name=f"outT{b}") for b in range(B)]
            vaug = {}  # (b, kc) -> [128, 2*(D+1)] tile
            for b in range(B):
                for kc in range(NK):
                    vaug[b, kc] = persist.tile([128, 2 * (D + 1)], BF16,
                                               tag=f"vaug{b}_{kc}",
                                               name=f"vaug{b}_{kc}")

            def phase_a(b):
                """QKV^T projection + V transpose for batch b."""
                # zero quadrants of QP (once per batch; stay zero)
                nc.vector.memset(QP[b][D:2 * D, 0:SEQ], 0.0)
                nc.vector.memset(QP[b][0:D, SEQ:2 * SEQ], 0.0)
                xts = None
                for sc in range(SEQ // 512):
                    s0 = sc * 512            # batch-local seq offset
                    g0 = b * SEQ + s0        # global column in xT
                    if sc % 2 == 0:
                        xts = []
                        for kc in range(CC):
                            t = xtp.tile([128, 1024], BF16, tag=f"xt{kc}")
                            nc.sync.dma_start(t[:], xT_c[kc, b * 2 + sc // 2])
                            xts.append(t)
                    xo = (sc % 2) * 512
                    for nm in ("q", "k", "v"):
                        ps = ps_misc.tile([128, 512], F32, tag="misc")
                        for kc in range(CC):
                            nc.tensor.matmul(ps[:], w_sb[nm, kc][:],
                                             xts[kc][:, xo:xo + 512],
                                             start=(kc == 0), stop=(kc == CC - 1))
                        if nm == "q":
                            for h in range(HPC):
                                nc.vector.tensor_scalar_add(
                                    QP[b][h * D:(h + 1) * D,
                                          h * SEQ + s0:h * SEQ + s0 + 512],
                                    ps[h * D:(h + 1) * D, :],
                                    bq_sb[h * D:(h + 1) * D, :])
                        elif nm == "k":
                            nc.vector.tensor_scalar_add(
                                KT[b][:, s0:s0 + 512], ps[:], bk_sb[:])
                        else:
                            vt = vtp.tile([128, 512], BF16, tag="vt")
                            nc.vector.tensor_scalar_add(vt[:], ps[:], bv_sb[:])
                            # transpose V^T -> V (natural), split per head with
                            # a ones column appended after each head's 64 cols
                            for j in range(4):
                                va = vaug[b, sc * 4 + j]
                                pst = ps_misc.tile([128, 128], BF16, tag="misc")
                                nc.tensor.transpose(
                                    pst[:], vt[:, j * 128:(j + 1) * 128],
                                    ident[:])
                                nc.vector.tensor_copy(va[:, 0:D], pst[:, 0:D])
                                nc.vector.tensor_copy(va[:, D + 1:2 * D + 1],
                                                      pst[:, D:2 * D])
                                nc.vector.tensor_copy(va[:, D:D + 1],
                                                      onescol_sb[:])
                                nc.vector.tensor_copy(
                                    va[:, 2 * D + 1:2 * D + 2], onescol_sb[:])

            def phase_bc(b):
                """Attention + output projection for batch b, interleaved
                per q-chunk so projection fills exp-gated PE slack."""
                for q in range(NQ):
                    q0 = q * QCH
                    pvs = [ps_pv.tile([D + 1, QCH], F32, tag=f"pv{h}",
                                      name=f"pv{h}") for h in range(HPC)]
                    for kc in range(NK):
                        st = ps_st.tile([128, 2 * QCH], F32, tag="st")
                        k0 = kc * 128
                        for h in range(HPC):
                            nc.tensor.matmul(
                                st[:, h * QCH:(h + 1) * QCH],
                                KT[b][:, k0:k0 + 128],
                                QP[b][:, h * SEQ + q0:h * SEQ + q0 + QCH],
                                start=True, stop=True)
                        pt = psb.tile([128, 2 * QCH], BF16, tag="pt")
                        nc.scalar.activation(pt[:], st[:],
                                             mybir.ActivationFunctionType.Exp,
                                             scale=SCALE)
                        for h in range(HPC):
                            nc.tensor.matmul(
                                pvs[h][:],
                                vaug[b, kc][:, h * (D + 1):(h + 1) * (D + 1)],
                                pt[:, h * QCH:(h + 1) * QCH],
                                start=(kc == 0), stop=(kc == NK - 1))
                    # normalize: out^T[d, q] / colsum -> outT (fp32r).
                    for h in range(HPC):
                        ss = normp.tile([1, QCH], F32R, tag="ss")
                        nc.vector.tensor_copy(ss[:], pvs[h][D:D + 1, :])
                        bc = ps_misc.tile([D, QCH], F32, tag="misc")
                        nc.tensor.matmul(bc[:], ones_sb[:], ss[:],
                                         start=True, stop=True)
                        rc = normp.tile([D, QCH], F32, tag="rc")
                        nc.vector.reciprocal_approx_fast(rc[:], bc[:])
                        nc.vector.tensor_mul(
                            outT[b][h * D:(h + 1) * D, q0:q0 + QCH],
                            pvs[h][0:D, :], rc[:])
                    # projection for the 4 seq-128 chunks of this q-chunk
                    for sc in range(4 * q, 4 * q + 4):
                        rt = b * (SEQ // 128) + sc
                        for n in range(EMB // 512):
                            ps = ps_misc.tile([128, 512], F32, tag="misc")
                            nc.tensor.matmul(
                                ps[:], outT[b][:, sc * 128:(sc + 1) * 128],
                                wout_sb[:, n * 512:(n + 1) * 512],
                                start=True, stop=True)
                            yt = youtp.tile([128, 512], F32, tag="yt")
                            nc.vector.tensor_copy(yt[:], ps[:])
                            nc.gpsimd.dma_start(y.ap()[rt, n], yt[:])

            phase_a(0)
            nc.sync.dma_start(wout_sb[:], wout.ap())
            phase_bc(0)
            phase_a(1)
            phase_bc(1)

    nc.compile()
    return nc


_NC = None


def _get_nc():
    global _NC
    if _NC is None:
        _NC = _build()
    return _NC


def kernel(x, W_qkv, b_qkv, W_out, b_out):
    x = np.asarray(x, dtype=np.float32)
    W_qkv = np.asarray(W_qkv, dtype=np.float32)
    b_qkv = np.asarray(b_qkv, dtype=np.float32)
    W_out = np.asarray(W_out, dtype=np.float32)
    b_out = np.asarray(b_out, dtype=np.float32)

    nc = _get_nc()

    xT2 = x.reshape(TSEQ, EMB).T.astype(NPBF16)      # [EMB, TSEQ]
    xT = np.ascontiguousarray(
        xT2.reshape(CC, 128, TSEQ // 1024, 1024).transpose(0, 2, 1, 3))
    Wr = W_qkv.reshape(EMB, 3, HEADS, D)
    br = b_qkv.reshape(3, HEADS, D)
    ones = np.ones((1, D), dtype=np.float32)
    onescol = np.ones((128, 1), dtype=np.float32)

    in_maps = []
    for c in range(NCORES):
        h0, h1 = HPC * c, HPC * (c + 1)
        in_maps.append({
            "xT": xT,
            "wq": np.ascontiguousarray(
                Wr[:, 0, h0:h1].reshape(EMB, LD)).astype(NPBF16),
            "wk": np.ascontiguousarray(
                Wr[:, 1, h0:h1].reshape(EMB, LD)).astype(NPBF16),
            "wv": np.ascontiguousarray(
                Wr[:, 2, h0:h1].reshape(EMB, LD)).astype(NPBF16),
            "bq": np.ascontiguousarray(br[0, h0:h1].reshape(LD, 1)),
            "bk": np.ascontiguousarray(br[1, h0:h1].reshape(LD, 1)),
            "bv": np.ascontiguousarray(br[2, h0:h1].reshape(LD, 1)),
            "wout": _round_fp32r(W_out[LD * c:LD * (c + 1)]),
            "ones": ones,
            "onescol": onescol,
        })

    res = bass_utils.run_bass_kernel_spmd(
        nc, in_maps, core_ids=list(range(NCORES)), trace=False)

    acc = np.zeros((TSEQ // 128, EMB // 512, 128, 512), dtype=np.float64)
    for c in range(NCORES):
        acc += res.results[c]["y"]
    yfull = acc.transpose(0, 2, 1, 3).reshape(TSEQ, EMB)
    out = (yfull + b_out).astype(np.float32)
    return out.reshape(B, SEQ, EMB)
